# revision 1
# baseline (speedup 1.0000x reference)
"""Trainium2 Bass kernel for nn_BGNN_MLP (bipartite 3-layer GNN).

Self-contained: kernel(**inputs) -> np.ndarray takes the full unsharded
inputs and returns the full [50000, 128] output, running on 8 NeuronCores
via run_bass_kernel_spmd.

Algorithm (per layer l = 0,1,2; directions U,V,U):
  z = input @ W_l            (dense, per-core slice, node-major)
  publish z slice -> AllGather -> Z table [8*WV, 128] in DRAM
  aggregate: out[d] = sum_{edges e: dest(e)=d} z[src(e)]  + deg(d)*b_l
    via per-superbin gather tiles (dma_gather, 128 edge slots/tile) and
    PE matmuls (gathered rows stationary, 0/1 selector M moving) into
    PSUM windows; the bias enters as a rank-1 outer(b, deg) matmul that
    also initializes each window.

SPMD: one instruction stream for all 8 cores; all per-core variation is
carried by ExternalInput data (packing layout, gather indices, M, deg).
"""

import sys

if "/opt/trn_rl_repo" not in sys.path:
    sys.path.insert(0, "/opt/trn_rl_repo")

import numpy as np

NC = 8

# ----------------------------------------------------------------------------
# host-side packing
# ----------------------------------------------------------------------------


def _pack_core(lo_cnt, hi_cnt, wm):
    """2D FFD, imbalance-aware. Returns list of bins (lists of local ids)."""
    order = np.argsort(-(lo_cnt + hi_cnt), kind="stable")
    bins, bl, bh = [], [], []
    open_bins = []
    for li in order:
        li = int(li)
        l, h = int(lo_cnt[li]), int(hi_cnt[li])
        best, best_score = -1, None
        for bi in open_bins:
            if len(bins[bi]) >= wm:
                continue
            nl, nh = bl[bi] + l, bh[bi] + h
            if nl > 128 or nh > 128:
                continue
            score = abs(nl - nh)
            if best_score is None or score < best_score:
                best_score, best = score, bi
        if best < 0:
            bins.append([li]); bl.append(l); bh.append(h)
        else:
            bins[best].append(li); bl[best] += l; bh[best] += h
        bi = best if best >= 0 else len(bins) - 1
        if bi not in open_bins:
            if not (max(bl[bi], bh[bi]) > 122 or len(bins[bi]) >= wm):
                open_bins.append(bi)
        elif max(bl[bi], bh[bi]) > 122 or len(bins[bi]) >= wm:
            open_bins.remove(bi)
        if len(open_bins) > 48:
            fullest = max(open_bins, key=lambda b2: max(bl[b2], bh[b2]))
            open_bins.remove(fullest)
    return bins


class DirPack:
    """Packing of one direction's dest space for all cores."""

    def __init__(self, dest, src, n, loc, wv, wm_try=24):
        self.n, self.loc, self.wv = n, loc, wv
        order = np.argsort(dest, kind="stable")
        self.dest_s = dest[order]
        self.src_s = src[order]
        counts = np.bincount(dest, minlength=n)
        self.starts = np.concatenate([[0], np.cumsum(counts)])
        src_core = self.src_s // loc
        self.lo_mask_s = src_core < 4

        wm = wm_try
        while True:
            max_bins = 0
            all_bins = []
            for c in range(NC):
                d0 = c * loc
                lo_cnt = np.zeros(loc, np.int64)
                hi_cnt = np.zeros(loc, np.int64)
                for li in range(loc):
                    s, e = self.starts[d0 + li], self.starts[d0 + li + 1]
                    lo = int(self.lo_mask_s[s:e].sum())
                    lo_cnt[li] = lo
                    hi_cnt[li] = (e - s) - lo
                bins = _pack_core(lo_cnt, hi_cnt, wm)
                max_bins = max(max_bins, len(bins))
                all_bins.append(bins)
            if max_bins * wm <= wv:
                break
            wm -= 1
            assert wm >= 12, "packing does not fit virtual slice"
        self.wm = wm
        self.wb = 512 // wm           # superbins per psum window
        self.nw = -(-max_bins // self.wb)
        self.nt = self.nw * self.wb   # uniform padded superbin count
        assert self.nt * wm <= wv
        self.width = self.nt * wm
        self.core_bins = all_bins

        self.vpos = np.zeros(n, np.int64)
        for c in range(NC):
            d0 = c * loc
            for b, members in enumerate(self.core_bins[c]):
                for j, li in enumerate(members):
                    self.vpos[d0 + li] = b * wm + j


def build_dir_data(packD: DirPack, packS: DirPack):
    """Per-core device arrays for one direction (packS gives src Z rows)."""
    wm, nt, width = packD.wm, packD.nt, packD.width
    loc, wv = packD.loc, packD.wv
    half = 4 * wv
    src = packD.src_s
    src_row_s = (src // loc) * wv + packS.vpos[src]

    cores = []
    for c in range(NC):
        d0 = c * loc
        idx_lo = np.zeros((nt, 128), np.int16)
        idx_hi = np.zeros((nt, 128), np.int16)
        m_lo = np.zeros((nt, 128, wm), np.float32)
        m_hi = np.zeros((nt, 128, wm), np.float32)
        deg = np.zeros(width, np.float32)
        for b, members in enumerate(packD.core_bins[c]):
            ptr_lo = ptr_hi = 0
            for j, li in enumerate(members):
                s, e = packD.starts[d0 + li], packD.starts[d0 + li + 1]
                deg[b * wm + j] = float(e - s)
                rows = src_row_s[s:e]
                lo = rows[rows < half]
                hi = rows[rows >= half]
                ur, um = np.unique(lo, return_counts=True)
                k = len(ur)
                idx_lo[b, ptr_lo:ptr_lo + k] = ur
                m_lo[b, ptr_lo:ptr_lo + k, j] = um
                ptr_lo += k
                ur, um = np.unique(hi, return_counts=True)
                k = len(ur)
                idx_hi[b, ptr_hi:ptr_hi + k] = ur - half
                m_hi[b, ptr_hi:ptr_hi + k, j] = um
                ptr_hi += k
            assert ptr_lo <= 128 and ptr_hi <= 128
        cores.append({"idx_lo": idx_lo, "idx_hi": idx_hi,
                      "m_lo": m_lo, "m_hi": m_hi, "deg": deg})
    return cores


def wrap_idx(idx_tiles):
    """[nt, 128] int16 -> SBUF wrapped layout [128, nt*8]."""
    nt = idx_tiles.shape[0]
    out = np.zeros((16, nt * 8), np.int16)
    for t in range(nt):
        out[:, 8 * t:8 * t + 8] = idx_tiles[t].reshape(8, 16).T
    return np.tile(out, (8, 1))


def m_flat(m_tiles):
    """[nt, 128, wm] -> [128, nt*wm] (slot on partitions)."""
    nt, _, wm = m_tiles.shape
    return m_tiles.transpose(1, 0, 2).reshape(128, nt * wm).copy()


def prepare_host_data(inputs, n, loc, wv):
    """All per-core ExternalInput arrays + structural params."""
    eu = np.asarray(inputs["edge_u"]).astype(np.int64)
    ev = np.asarray(inputs["edge_v"]).astype(np.int64)
    X_v = np.asarray(inputs["X_v"], dtype=np.float32)

    packU = DirPack(eu, ev, n, loc, wv)   # dest u (layers 0, 2)
    packV = DirPack(ev, eu, n, loc, wv)   # dest v (layer 1)
    dataU = build_dir_data(packU, packV)
    dataV = build_dir_data(packV, packU)

    w_all = np.concatenate(
        [np.asarray(inputs[f"W{i}"], np.float32) for i in range(3)], axis=1)
    bias = np.zeros((16, 384), np.float32)
    for i in range(3):
        bias[0, 128 * i:128 * (i + 1)] = np.asarray(inputs[f"b{i}"], np.float32)

    per_core = []
    for c in range(NC):
        xT = np.zeros((128, packV.width), np.float32)
        g = np.arange(c * loc, (c + 1) * loc)
        xT[:, packV.vpos[g]] = X_v[g].T
        degu = np.zeros((16, packU.width), np.float32)
        degu[0] = dataU[c]["deg"]
        degv = np.zeros((16, packV.width), np.float32)
        degv[0] = dataV[c]["deg"]
        per_core.append({
            "xT": xT,
            "w_all": w_all,
            "bias": bias,
            "degu": degu,
            "degv": degv,
            "idxu_lo": wrap_idx(dataU[c]["idx_lo"]),
            "idxu_hi": wrap_idx(dataU[c]["idx_hi"]),
            "idxv_lo": wrap_idx(dataV[c]["idx_lo"]),
            "idxv_hi": wrap_idx(dataV[c]["idx_hi"]),
            "mu_lo": m_flat(dataU[c]["m_lo"]),
            "mu_hi": m_flat(dataU[c]["m_hi"]),
            "mv_lo": m_flat(dataV[c]["m_lo"]),
            "mv_hi": m_flat(dataV[c]["m_hi"]),
        })
    return packU, packV, per_core


# ----------------------------------------------------------------------------
# walrus drain workaround: split multi-wait tail Drain into single-wait nops
# ----------------------------------------------------------------------------


def _patch_tile_drain():
    from concourse import tile
    if getattr(tile.TileContext, "_bgnn_drain_patched", False):
        return
    from concourse.vector_clock import ScopedClock

    def patched(self, tick_clock, wait_clock):
        nc = self.nc
        nops = [nc.sync.nop() for _ in range(31)]
        drain_inst = nc.sync.drain()
        wait_clock.add_sem_waits(
            drain_inst.ins, ScopedClock({None: tick_clock.global_clock})
        )
        si = drain_inst.ins.sync_info
        waits = list(si.on_wait) if si is not None else []
        if len(waits) > 1:
            assert len(waits) - 1 <= len(nops), len(waits)
            for i, w in enumerate(waits[:-1]):
                n = nops[i].ins
                nsi = n.sync_info
                if nsi is None:
                    n.sync_info = type(si)(on_wait=[w], on_update=[])
                else:
                    nsi.on_wait = list(nsi.on_wait) + [w]
            si.on_wait = waits[-1:]
        nc.all_engine_barrier()
        popped = nc._tile_sem_poison_stack.pop()
        assert popped is self._sem_poison
        nc.clear_and_free_semaphores(list(self.sems.allocated().values()))
        nc.all_engine_barrier()

    tile.TileContext._drain_and_barrier = patched
    tile.TileContext._bgnn_drain_patched = True


# ----------------------------------------------------------------------------
# device program
# ----------------------------------------------------------------------------


def build_program(packU: DirPack, packV: DirPack, max_steps: int = 99):
    """max_steps: debug truncation. Each layer = 3 steps (dense, AG, agg)."""
    import concourse.bass as bass
    import concourse.mybir as mybir
    from concourse import bacc, tile

    _patch_tile_drain()
    f32 = mybir.dt.float32
    i16 = mybir.dt.int16

    wv = packU.wv
    zrows = NC * wv
    half = 4 * wv
    widthU, widthV = packU.width, packV.width
    wmax = max(widthU, widthV)

    nc = bacc.Bacc(num_swdge_queues=4)
    core_ids = list(range(NC))

    # I/O
    xT_d = nc.dram_tensor("xT", [128, widthV], f32, kind="ExternalInput")
    w_d = nc.dram_tensor("w_all", [128, 384], f32, kind="ExternalInput")
    bias_d = nc.dram_tensor("bias", [16, 384], f32, kind="ExternalInput")
    degu_d = nc.dram_tensor("degu", [16, widthU], f32, kind="ExternalInput")
    degv_d = nc.dram_tensor("degv", [16, widthV], f32, kind="ExternalInput")
    idx_d = {}
    m_d = {}
    for dirn, pk in (("u", packU), ("v", packV)):
        for s in ("lo", "hi"):
            idx_d[dirn, s] = nc.dram_tensor(
                f"idx{dirn}_{s}", [128, pk.nt * 8], i16, kind="ExternalInput")
            m_d[dirn, s] = nc.dram_tensor(
                f"m{dirn}_{s}", [128, pk.nt * pk.wm], f32, kind="ExternalInput")
    out_d = nc.dram_tensor("outp", [128, widthU], f32, kind="ExternalOutput")

    # internal DRAM
    z_d = nc.dram_tensor("z_slice", [wv, 128], f32)
    zfull_d = nc.dram_tensor("z_full", [zrows, 128], f32, addr_space="Shared")

    layers = [
        ("u", packU, degu_d, 0),   # layer 0: dense over V layout, agg to U
        ("v", packV, degv_d, 1),   # layer 1: dense over U layout, agg to V
        ("u", packU, degu_d, 2),   # layer 2: dense over V layout, agg to U
    ]

    with tile.TileContext(nc) as tc:
        with (
            tc.tile_pool(name="persist", bufs=1) as persist,
            tc.tile_pool(name="zstage", bufs=2) as zstage_pool,
            tc.tile_pool(name="glo", bufs=2) as glo_pool,
            tc.tile_pool(name="ghi", bufs=2) as ghi_pool,
            tc.tile_pool(name="mslab", bufs=3) as m_pool,
            tc.tile_pool(name="degslab", bufs=2) as deg_pool,
            tc.tile_pool(name="pagg", bufs=2, space="PSUM") as pagg_pool,
            tc.tile_pool(name="pdense", bufs=4, space="PSUM") as pdense_pool,
        ):
            inA = persist.tile([128, wmax], f32, tag="inA")
            inB = persist.tile([128, wmax], f32, tag="inB")
            if max_steps < 9:
                nc.vector.memset(inB[:], 0.0)
                nc.vector.memset(inA[:], 0.0)
            w_sb = persist.tile([128, 384], f32, tag="w")
            bias_sb = persist.tile([16, 384], f32, tag="bias")
            idx_sb = {}
            for dirn, pk in (("u", packU), ("v", packV)):
                for s in ("lo", "hi"):
                    idx_sb[dirn, s] = persist.tile(
                        [128, pk.nt * 8], i16, tag=f"idx{dirn}{s}",
                        name=f"idx{dirn}{s}")

            # preload
            nc.sync.dma_start(out=inA[:, 0:widthV], in_=xT_d[:])
            nc.sync.dma_start(out=w_sb[:], in_=w_d[:])
            nc.sync.dma_start(out=bias_sb[:], in_=bias_d[:])
            for key, t in idx_sb.items():
                nc.sync.dma_start(out=t[:], in_=idx_d[key][:])

            gather_regs = {}
            gather_call_no = [0]

            bufs = [inA, inB]
            step = 0
            for li, (dirn, pk, deg_d, wl) in enumerate(layers):
                if step >= max_steps:
                    break
                step += 1
                src_pk = packV if dirn == "u" else packU
                swidth = src_pk.width
                cur_in = bufs[li % 2]
                dst = bufs[(li + 1) % 2]

                # ---- dense: z[n, :] = in.T @ W_l, staged + 2 DMAs ----
                nch = -(-swidth // 128)
                half_ch = -(-nch // 2)
                for stg in range(2):
                    k0 = stg * half_ch
                    k1 = min(nch, (stg + 1) * half_ch)
                    if k0 >= k1:
                        continue
                    zst = zstage_pool.tile([128, half_ch * 128], f32, tag="zst")
                    if k1 * 128 > swidth:
                        w_last = swidth - (k1 - 1) * 128
                        p0 = (w_last // 32) * 32  # start partition must be 32-aligned
                        blk = slice((k1 - 1 - k0) * 128, (k1 - k0) * 128)
                        for q in range(p0, 128, 32):
                            # non-zero-base SBUF APs may span at most 32 parts
                            nc.vector.memset(zst[q:q + 32, blk], 0.0)
                    for k in range(k0, k1):
                        w = min(128, swidth - k * 128)
                        pz = pdense_pool.tile([128, 128], f32, tag="pz")
                        nc.tensor.matmul(
                            pz[0:w, :],
                            lhsT=cur_in[:, k * 128:k * 128 + w],
                            rhs=w_sb[:, wl * 128:(wl + 1) * 128],
                            start=True, stop=True,
                        )
                        nc.vector.tensor_copy(
                            zst[0:w, (k - k0) * 128:(k - k0) * 128 + 128],
                            pz[0:w, :])
                    # SBUF [p, k, f] -> DRAM rows k*128+p
                    n_k = k1 - k0
                    src_ap = zst[:, 0:n_k * 128].rearrange(
                        "p (k f) -> p k f", f=128)
                    dst_ap = z_d.rearrange(
                        "(kk p) f -> p kk f", p=128)[:, k0:k1, :]
                    nc.sync.dma_start(out=dst_ap, in_=src_ap)

                # ---- all-gather ----
                if step >= max_steps:
                    break
                step += 1
                nc.gpsimd.collective_compute(
                    "AllGather",
                    mybir.AluOpType.bypass,
                    replica_groups=[core_ids],
                    ins=[z_d[:]],
                    outs=[zfull_d[:]],
                )

                # ---- aggregation ----
                if step >= max_steps:
                    break
                step += 1
                wm, wb, nw = pk.wm, pk.wb, pk.nw
                wcols = wb * wm
                for w in range(nw):
                    g_lo = glo_pool.tile([128, wb * 128], f32, tag="glo")
                    g_hi = ghi_pool.tile([128, wb * 128], f32, tag="ghi")
                    # split into <=8-tile chunks: single_packet descriptor
                    # generation is ~20x faster but breaks above ~1024 idxs
                    for c0 in range(0, wb, 8):
                        c1 = min(wb, c0 + 8)
                        nci = (c1 - c0) * 128
                        if nci not in gather_regs:
                            gather_regs[nci] = nc.gpsimd.to_reg(nci)
                        for stream, gbuf in (("lo", g_lo), ("hi", g_hi)):
                            src = (zfull_d[0:half, :] if stream == "lo"
                                   else zfull_d[half:2 * half, :])
                            nc.gpsimd.dma_gather(
                                gbuf[:, c0 * 128:c1 * 128].rearrange(
                                    "p (t e) -> p t e", e=128),
                                src,
                                idx_sb[dirn, stream][
                                    :, (w * wb + c0) * 8:(w * wb + c1) * 8],
                                num_idxs=nci,
                                num_idxs_reg=gather_regs[nci],
                                elem_size=128,
                                queue_num=gather_call_no[0] % 4,
                            )
                            gather_call_no[0] += 1
                    m_lo = m_pool.tile([128, wcols], f32, tag="mlo")
                    m_hi = m_pool.tile([128, wcols], f32, tag="mhi")
                    nc.sync.dma_start(
                        out=m_lo[:], in_=m_d[dirn, "lo"][:, w * wcols:(w + 1) * wcols])
                    nc.sync.dma_start(
                        out=m_hi[:], in_=m_d[dirn, "hi"][:, w * wcols:(w + 1) * wcols])
                    deg_sl = deg_pool.tile([16, wcols], f32, tag="deg")
                    nc.sync.dma_start(
                        out=deg_sl[:], in_=deg_d[0:16, w * wcols:(w + 1) * wcols])

                    pw = pagg_pool.tile([128, wcols], f32, tag="pagg")
                    nc.tensor.matmul(
                        pw[:],
                        lhsT=bias_sb[0:16, wl * 128:(wl + 1) * 128],
                        rhs=deg_sl[:],
                        start=True, stop=False, skip_group_check=True,
                    )
                    for t in range(wb):
                        nc.tensor.matmul(
                            pw[:, t * wm:(t + 1) * wm],
                            lhsT=g_lo[:, t * 128:(t + 1) * 128],
                            rhs=m_lo[:, t * wm:(t + 1) * wm],
                            start=False, stop=False, skip_group_check=True,
                        )
                    for t in range(wb):
                        nc.tensor.matmul(
                            pw[:, t * wm:(t + 1) * wm],
                            lhsT=g_hi[:, t * 128:(t + 1) * 128],
                            rhs=m_hi[:, t * wm:(t + 1) * wm],
                            start=False, stop=(t == wb - 1),
                            skip_group_check=True,
                        )
                    nc.vector.tensor_copy(
                        dst[:, w * wcols:(w + 1) * wcols], pw[:])

            # output: last agg landed in bufs[3 % 2] = inB
            nc.sync.dma_start(out=out_d[:], in_=bufs[1][:, 0:widthU])

    nc.compile()
    return nc


# ----------------------------------------------------------------------------
# public entry point
# ----------------------------------------------------------------------------


def kernel(**inputs) -> np.ndarray:
    from concourse.bass_utils import run_bass_kernel_spmd

    n = int(np.asarray(inputs["X_u"]).shape[0])
    loc = n // NC
    # wv: virtual rows per core slice; 4*wv must be >= any lo/hi idx range
    wv = 8192 if n == 50000 else max(512, 1 << (loc * 2 - 1).bit_length())

    packU, packV, per_core = prepare_host_data(inputs, n, loc, wv)
    nc = build_program(packU, packV)
    res = run_bass_kernel_spmd(nc, per_core, list(range(NC)))

    out = np.zeros((n, 128), np.float32)
    for c in range(NC):
        g = np.arange(c * loc, (c + 1) * loc)
        out[g] = res.results[c]["outp"][:, packU.vpos[g]].T
    return out


if __name__ == "__main__":
    data = dict(np.load("/root/problem/inputs_cache.npz"))
    got = kernel(**data)
    np.save("/root/problem/kernel_out.npy", got)
    print("kernel done", got.shape)



# revision 4
# speedup vs baseline: 3.2389x; 3.2389x over previous
"""Trainium2 Bass kernel for nn_BGNN_MLP (bipartite 3-layer GNN).

Self-contained: kernel(**inputs) -> np.ndarray takes the full unsharded
inputs and returns the full [50000, 128] output, running on 8 NeuronCores
via run_bass_kernel_spmd.

Key algebraic restructuring: the Linear layers commute with the (linear)
segment-sum aggregations, so

  out = A2 A1 A0 X_v W012 + (A2 A1 deg0) c2^T + (A2 deg1) c1^T + deg2 c0^T

with W012 = W0 W1 W2, c2 = (W1 W2)^T b0, c1 = W2^T b1, c0 = b2, and
Ai the per-layer aggregation matrices (A0=A2=v2u, A1=u2v). The deg
vectors are pure graph structure, computed on host; W012/c* are computed
on device in a tiny preamble. The device mainline is then just three
sparse aggregations over 128-feature node tables:

  per layer: gather rows of the (AllGathered, bf16) source table via
  dma_gather into 128-slot tiles; one PE matmul per tile with a [128,96]
  0/1 selector as the stationary operand accumulates each tile into a
  96-destination "page" of a PSUM bank ([dest, feat] layout, fp32); the
  bank is copied (cast to bf16) into a staging slab, DMA'd out in a
  handful of large contiguous descriptors, and AllGathered in 4 chunks
  that overlap the remaining windows.

SPMD: one instruction stream for all 8 cores; all per-core variation is
carried by ExternalInput data (positions, gather indices, M selectors).
"""

import sys

if "/opt/trn_rl_repo" not in sys.path:
    sys.path.insert(0, "/opt/trn_rl_repo")

import numpy as np

NC = 8
D = 128
PAGE = 96          # destinations per PSUM page (MM output partitions)
PAGES = 4          # pages per PSUM bank (free-dim 512B each)
WINDOW = PAGE * PAGES  # 384 dests per window
SUBS = 4           # AllGather chunks per table

# ----------------------------------------------------------------------------
# host-side packing
# ----------------------------------------------------------------------------


def snake_positions(deg_local, nw):
    """Assign local dests to (window, page, slot) balancing rows per page.

    Returns pos[local_id] in [0, nw*WINDOW). Buckets = nw*PAGES pages,
    each gets <= PAGE dests, snake-dealt by descending degree.
    """
    loc = len(deg_local)
    nbuck = nw * PAGES
    order = np.argsort(-deg_local, kind="stable")
    pos = np.zeros(loc, np.int64)
    fill = np.zeros(nbuck, np.int64)
    b = 0
    direction = 1
    for i, d in enumerate(order):
        # snake over buckets
        tries = 0
        while fill[b] >= PAGE:
            b += direction
            if b == nbuck or b < 0:
                direction = -direction
                b += direction
            tries += 1
            assert tries <= 2 * nbuck
        pos[d] = b * PAGE + fill[b]
        fill[b] += 1
        b += direction
        if b == nbuck or b < 0:
            direction = -direction
            b += direction
    return pos


class DirPack:
    """One direction: dest packing + per-core tile/idx/M data."""

    def __init__(self, dest, src, n, loc, nw, src_row_of, pos):
        """src_row_of: [n] -> global row in the source table; pos: [n] ->
        local position of each dest within its core's table."""
        self.n, self.loc, self.nw = n, loc, nw
        self.R = nw * WINDOW
        self.pos = pos

        # edges sorted by dest
        order = np.argsort(dest, kind="stable")
        src_s = src[order]
        starts = np.concatenate([[0], np.cumsum(np.bincount(dest, minlength=n))])
        self.srow_s = src_row_of[src_s]
        self.starts = starts

    def build_tiles(self, half):
        """Compute per-(core, window, page, stream) tiles.

        Returns: T[w][pg][s] uniform tile counts, and per-core slot data.
        """
        n, loc, nw = self.n, self.loc, self.nw
        starts, srow_s = self.starts, self.srow_s
        # per-core, per-page, per-stream row lists (in position order)
        # rows_data[c][(w,pg,s)] = (list of (srcrow, slot_pos_in_page))
        core_rows = []
        cnt = np.zeros((NC, nw, PAGES, 2), np.int64)
        for c in range(NC):
            d0 = c * loc
            per_page = [[[ [], [] ] for _ in range(PAGES)] for _ in range(nw)]
            for li in range(loc):
                node = d0 + li
                q = self.pos[node]
                w, r = divmod(q, WINDOW)
                pg, p = divmod(r, PAGE)
                s0, e0 = starts[node], starts[node + 1]
                rows = srow_s[s0:e0]
                lo = rows[rows < half]
                hi = rows[rows >= half] - half
                if len(lo):
                    per_page[w][pg][0].append((p, lo))
                if len(hi):
                    per_page[w][pg][1].append((p, hi))
            for w in range(nw):
                for pg in range(PAGES):
                    for s in range(2):
                        tot = sum(len(r) for _, r in per_page[w][pg][s])
                        cnt[c, w, pg, s] = tot
            core_rows.append(per_page)
        # uniform tile counts
        T = np.ceil(cnt.max(axis=0) / 128).astype(np.int64)  # [nw, PAGES, 2]
        self.T = T
        self.core_rows = core_rows
        return T

    def emit_slabs(self, half):
        """Build idx + M slabs per core following the uniform schedule.

        Tile order (global): for w, for s in (lo, hi), for pg, for t.
        Gather calls: per (w, s): tiles of all pages consecutively, split
        into chunks of 8.
        """
        nw = self.nw
        T = self.T
        ntiles = int(T.sum())
        self.ntiles = ntiles
        # per-window tile counts per stream
        self.wtiles = [
            [int(T[w, :, s].sum()) for s in range(2)] for w in range(nw)
        ]
        idx_all = []
        m_all = []
        for c in range(NC):
            idx = np.zeros((ntiles, 128), np.int64)
            M = np.zeros((ntiles, 128, PAGE), np.float32)
            ti = 0
            per_page = self.core_rows[c]
            for w in range(nw):
                for s in range(2):
                    for pg in range(PAGES):
                        nt = int(T[w, pg, s])
                        if nt == 0:
                            continue
                        # flatten this page-stream's rows
                        prs = []
                        for p, rows in per_page[w][pg][s]:
                            for r in rows:
                                prs.append((r, p))
                        assert len(prs) <= nt * 128
                        for j, (r, p) in enumerate(prs):
                            t, sl = divmod(j, 128)
                            idx[ti + t, sl] = r
                            M[ti + t, sl, p] = 1.0
                        ti += nt
            assert ti == ntiles
            idx_all.append(idx)
            m_all.append(M)
        self.idx_all = idx_all
        self.m_all = m_all


def wrap_idx_calls(idx_tiles, wtiles):
    """Build the wrapped idx slab: per (window, stream) calls of <=8 tiles.

    idx_tiles: [ntiles, 128] in global tile order (w, s, pg, t).
    Returns [128, ncalls*64] int16 and the call schedule
    [(num_tiles, tile_base), ...].
    """
    schedule = []
    base = 0
    for w in range(len(wtiles)):
        for s in range(2):
            nt = wtiles[w][s]
            t0 = 0
            while t0 < nt:
                k = min(8, nt - t0)
                schedule.append((k, base + t0))
                t0 += k
            base += nt
    ncalls = len(schedule)
    out = np.zeros((16, ncalls * 64), np.int16)
    for ci, (k, tb) in enumerate(schedule):
        flat = idx_tiles[tb:tb + k].reshape(-1)  # k*128
        wrapped = flat.reshape(-1, 16).T  # [16, k*8]
        out[:, ci * 64: ci * 64 + k * 8] = wrapped
    return np.tile(out, (8, 1)), schedule


def bf16(x):
    """fp32 -> bf16 (round-to-nearest-even) stored as uint16 view."""
    x = np.asarray(x, np.float32)
    u = x.view(np.uint32)
    r = ((u >> 16) & 1) + 0x7FFF
    return ((u + r) >> 16).astype(np.uint16)


def prepare_host_data(inputs):
    eu = np.asarray(inputs["edge_u"]).astype(np.int64)
    ev = np.asarray(inputs["edge_v"]).astype(np.int64)
    X_v = np.asarray(inputs["X_v"], dtype=np.float32)
    n = int(np.asarray(inputs["X_u"]).shape[0])
    loc = n // NC
    nw = -(-loc // WINDOW)
    R = nw * WINDOW
    RG = R * NC
    half = RG // 2

    # AG chunk groups: SUBS groups of whole windows
    wgroups = np.array_split(np.arange(nw), SUBS)
    grows = np.array([len(wg) * WINDOW for wg in wgroups])      # local rows
    gcum = np.concatenate([[0], np.cumsum(grows)])              # local offsets
    ggcum = np.concatenate([[0], np.cumsum(grows * NC)])        # global offsets
    assert gcum[-1] == R and ggcum[-1] == RG

    def glob_rows(pos):
        """local position [n] -> global table row, group-chunked AG layout."""
        grp = np.searchsorted(gcum, pos, side="right") - 1
        cores = np.arange(n) // loc
        return ggcum[grp] + cores * grows[grp] + (pos - gcum[grp])

    # --- position maps for both dest spaces ---
    degU = np.bincount(eu, minlength=n)
    degV = np.bincount(ev, minlength=n)
    posU = np.zeros(n, np.int64)
    posV = np.zeros(n, np.int64)
    for c in range(NC):
        g = slice(c * loc, (c + 1) * loc)
        posU[g] = snake_positions(degU[g], nw)
        posV[g] = snake_positions(degV[g], nw)
    rowU = glob_rows(posU)
    rowV = glob_rows(posV)

    # packU: dests U (layers 0,2), sources V; packV: dests V (layer 1)
    packU = DirPack(eu, ev, n, loc, nw, rowV, posU)
    packU.build_tiles(half)
    packU.emit_slabs(half)

    packV = DirPack(ev, eu, n, loc, nw, rowU, posV)
    packV.build_tiles(half)
    packV.emit_slabs(half)

    # --- deg aggregation vectors (host, structure only) ---
    deg0 = degU.astype(np.float64)   # A0 * 1  (per U node)
    deg1 = degV.astype(np.float64)   # A1 * 1  (per V node)
    t_v = np.bincount(ev, weights=deg0[eu], minlength=n)   # A1 deg0 (per V)
    v2 = np.bincount(eu, weights=t_v[ev], minlength=n)     # A2 A1 deg0 (per U)
    v1 = np.bincount(eu, weights=deg1[ev], minlength=n)    # A2 deg1 (per U)
    v0 = degU.astype(np.float64)                           # deg2 (per U)

    # --- per-core external inputs ---
    W0 = np.asarray(inputs["W0"], np.float32)
    W1 = np.asarray(inputs["W1"], np.float32)
    W2 = np.asarray(inputs["W2"], np.float32)
    b0 = np.asarray(inputs["b0"], np.float32)
    b1 = np.asarray(inputs["b1"], np.float32)
    b2 = np.asarray(inputs["b2"], np.float32)

    bmat = np.zeros((128, 32), np.float32)
    bmat[:, 0] = b0
    bmat[:, 17] = b1
    cbase = np.zeros((16, 128), np.float32)
    cbase[2] = b2

    per_core = []
    for c in range(NC):
        g = np.arange(c * loc, (c + 1) * loc)
        xT = np.zeros((128, R), np.float32)
        xT[:, posV[g]] = X_v[g].T

        V3 = np.zeros((16, R), np.float32)
        V3[0, posU[g]] = v2[g]
        V3[1, posU[g]] = v1[g]
        V3[2, posU[g]] = v0[g]

        idxU_w, schedU = wrap_idx_calls(packU.idx_all[c], packU.wtiles)
        idxV_w, schedV = wrap_idx_calls(packV.idx_all[c], packV.wtiles)

        mU = packU.m_all[c].transpose(1, 0, 2).reshape(128, -1)  # [128, ntiles*96]
        mV = packV.m_all[c].transpose(1, 0, 2).reshape(128, -1)

        per_core.append({
            "xT": bf16(xT),
            "V3": V3,
            "idxU": idxU_w,
            "idxV": idxV_w,
            "mU": bf16(mU),
            "mV": bf16(mV),
            "W0T": W0.T.copy(),
            "W1T": W1.T.copy(),
            "W2": W2.copy(),
            "bmat": bmat,
            "cbase": cbase,
        })

    meta = {
        "n": n, "loc": loc, "nw": nw, "R": R, "RG": RG, "half": half,
        "sub_rows": sub_rows,
        "schedU": schedU, "schedV": schedV,
        "packU": packU, "packV": packV,
        "posU": posU,
    }
    return meta, per_core


# ----------------------------------------------------------------------------
# walrus drain workaround: split multi-wait tail Drain into single-wait nops
# ----------------------------------------------------------------------------


def _patch_tile_drain():
    from concourse import tile
    if getattr(tile.TileContext, "_bgnn_drain_patched", False):
        return
    from concourse.vector_clock import ScopedClock

    def patched(self, tick_clock, wait_clock):
        nc = self.nc
        nops = [nc.sync.nop() for _ in range(31)]
        drain_inst = nc.sync.drain()
        wait_clock.add_sem_waits(
            drain_inst.ins, ScopedClock({None: tick_clock.global_clock})
        )
        si = drain_inst.ins.sync_info
        waits = list(si.on_wait) if si is not None else []
        if len(waits) > 1:
            assert len(waits) - 1 <= len(nops), len(waits)
            for i, w in enumerate(waits[:-1]):
                n = nops[i].ins
                nsi = n.sync_info
                if nsi is None:
                    n.sync_info = type(si)(on_wait=[w], on_update=[])
                else:
                    nsi.on_wait = list(nsi.on_wait) + [w]
            si.on_wait = waits[-1:]
        nc.all_engine_barrier()
        popped = nc._tile_sem_poison_stack.pop()
        assert popped is self._sem_poison
        nc.clear_and_free_semaphores(list(self.sems.allocated().values()))
        nc.all_engine_barrier()

    tile.TileContext._drain_and_barrier = patched
    tile.TileContext._bgnn_drain_patched = True


# ----------------------------------------------------------------------------
# device program
# ----------------------------------------------------------------------------


def build_program(meta, max_layers=3):
    import concourse.bass as bass
    import concourse.mybir as mybir
    from concourse import bacc, tile

    _patch_tile_drain()
    f32 = mybir.dt.float32
    bf = mybir.dt.bfloat16
    i16 = mybir.dt.int16

    nw, R, RG, half = meta["nw"], meta["R"], meta["RG"], meta["half"]
    sub_rows = meta["sub_rows"]
    packU, packV = meta["packU"], meta["packV"]
    schedU, schedV = meta["schedU"], meta["schedV"]
    ntU, ntV = packU.ntiles, packV.ntiles
    ncallU, ncallV = len(schedU), len(schedV)

    nc = bacc.Bacc(num_swdge_queues=4)
    core_ids = list(range(NC))

    # I/O
    xT_d = nc.dram_tensor("xT", [128, R], bf, kind="ExternalInput")
    V3_d = nc.dram_tensor("V3", [16, R], f32, kind="ExternalInput")
    idxU_d = nc.dram_tensor("idxU", [128, ncallU * 64], i16, kind="ExternalInput")
    idxV_d = nc.dram_tensor("idxV", [128, ncallV * 64], i16, kind="ExternalInput")
    mU_d = nc.dram_tensor("mU", [128, ntU * PAGE], bf, kind="ExternalInput")
    mV_d = nc.dram_tensor("mV", [128, ntV * PAGE], bf, kind="ExternalInput")
    W0T_d = nc.dram_tensor("W0T", [128, 128], f32, kind="ExternalInput")
    W1T_d = nc.dram_tensor("W1T", [128, 128], f32, kind="ExternalInput")
    W2_d = nc.dram_tensor("W2", [128, 128], f32, kind="ExternalInput")
    bmat_d = nc.dram_tensor("bmat", [128, 32], f32, kind="ExternalInput")
    cbase_d = nc.dram_tensor("cbase", [16, 128], f32, kind="ExternalInput")
    out_d = nc.dram_tensor("outp", [R, 128], f32, kind="ExternalOutput")

    # internal DRAM
    zP_d = nc.dram_tensor("zP", [R, 128], bf)
    z0_d = nc.dram_tensor("z0", [R, 128], bf)
    z1_d = nc.dram_tensor("z1", [R, 128], bf)
    tabV_d = nc.dram_tensor("tabV", [RG, 128], bf, addr_space="Shared")
    tabU_d = nc.dram_tensor("tabU", [RG, 128], bf, addr_space="Shared")

    layers = [
        # (pack, sched, idx_d, m_d, src_tab, out z slice, out tab or None)
        (packU, schedU, idxU_d, mU_d, tabV_d, z0_d, tabU_d),
        (packV, schedV, idxV_d, mV_d, tabU_d, z1_d, tabV_d),
        (packU, schedU, idxU_d, mU_d, tabV_d, None, None),
    ]

    with tile.TileContext(nc) as tc:
        with (
            tc.tile_pool(name="persist", bufs=1) as persist,
            tc.tile_pool(name="g", bufs=8) as g_pool,
            tc.tile_pool(name="mslab", bufs=2) as m_pool,
            tc.tile_pool(name="pagg", bufs=3, space="PSUM") as pagg_pool,
            tc.tile_pool(name="pdense", bufs=2, space="PSUM") as pdense_pool,
        ):
            xT_sb = persist.tile([128, R], bf, tag="xT")
            V3_sb = persist.tile([16, R], f32, tag="V3")
            idxU_sb = persist.tile([128, ncallU * 64], i16, tag="idxU")
            idxV_sb = persist.tile([128, ncallV * 64], i16, tag="idxV")
            w0t_sb = persist.tile([128, 128], f32, tag="w0t")
            w1t_sb = persist.tile([128, 128], f32, tag="w1t")
            w2_sb = persist.tile([128, 128], f32, tag="w2")
            bmat_sb = persist.tile([128, 32], f32, tag="bmat")
            cbase_sb = persist.tile([16, 128], f32, tag="cbase")
            w12_sb = persist.tile([128, 128], f32, tag="w12")
            w012_sb = persist.tile([128, 128], bf, tag="w012")
            C_sb = persist.tile([16, 128], f32, tag="C")
            stag = persist.tile([128, (R // PAGE) * 128], bf, tag="stag")
            stag32 = persist.tile([128, (R // PAGE) * 128], f32, tag="stag32")

            # preload
            nc.sync.dma_start(out=xT_sb[:], in_=xT_d[:])
            nc.sync.dma_start(out=V3_sb[:], in_=V3_d[:])
            nc.sync.dma_start(out=idxU_sb[:], in_=idxU_d[:])
            nc.sync.dma_start(out=idxV_sb[:], in_=idxV_d[:])
            nc.sync.dma_start(out=w0t_sb[:], in_=W0T_d[:])
            nc.sync.dma_start(out=w1t_sb[:], in_=W1T_d[:])
            nc.sync.dma_start(out=w2_sb[:], in_=W2_d[:])
            nc.sync.dma_start(out=bmat_sb[:], in_=bmat_d[:])
            nc.sync.dma_start(out=cbase_sb[:], in_=cbase_d[:])

            # ---- preamble: W12 = W1@W2; W012 = W0@W12; C rows ----
            p1 = pdense_pool.tile([128, 128], f32, tag="pw")
            nc.tensor.matmul(p1[:], lhsT=w1t_sb[:], rhs=w2_sb[:],
                             start=True, stop=True)
            nc.vector.tensor_copy(w12_sb[:], p1[:])
            p2 = pdense_pool.tile([128, 128], f32, tag="pw")
            nc.tensor.matmul(p2[:], lhsT=w0t_sb[:], rhs=w12_sb[:],
                             start=True, stop=True)
            nc.vector.tensor_copy(w012_sb[:], p2[:])
            pc = pdense_pool.tile([16, 128], f32, tag="pc")
            nc.tensor.matmul(pc[:], lhsT=bmat_sb[:, 0:16], rhs=w12_sb[:],
                             start=True, stop=False, skip_group_check=True)
            nc.tensor.matmul(pc[:], lhsT=bmat_sb[:, 16:32], rhs=w2_sb[:],
                             start=False, stop=True, skip_group_check=True)
            nc.vector.tensor_tensor(
                out=C_sb[:], in0=cbase_sb[:], in1=pc[:],
                op=mybir.AluOpType.add)

            # ---- dense: zP = (X W012) rows at V positions ----
            nchunk = R // PAGE
            for k in range(nchunk):
                pz = pdense_pool.tile([128, 128], f32, tag="pz")
                nc.tensor.matmul(
                    pz[0:PAGE, :],
                    lhsT=xT_sb[:, k * PAGE:(k + 1) * PAGE],
                    rhs=w012_sb[:],
                    start=True, stop=True,
                )
                nc.vector.tensor_copy(
                    stag[0:PAGE, k * 128:(k + 1) * 128], pz[0:PAGE, :])
            # z-write + AG chunks
            for sc in range(SUBS):
                k0 = sc * (nchunk // SUBS)
                k1 = (sc + 1) * (nchunk // SUBS)
                dst = zP_d.rearrange("(k p) f -> p k f", p=PAGE)[:, k0:k1, :]
                src = stag[0:PAGE, k0 * 128:k1 * 128].rearrange(
                    "p (k f) -> p k f", f=128)
                nc.sync.dma_start(out=dst, in_=src)
                nc.gpsimd.collective_compute(
                    "AllGather", mybir.AluOpType.bypass,
                    replica_groups=[core_ids],
                    ins=[zP_d[sc * sub_rows:(sc + 1) * sub_rows, :]],
                    outs=[tabV_d[sc * sub_rows * NC:(sc + 1) * sub_rows * NC, :]],
                )

            gather_regs = {}
            call_no = [0]

            for li in range(max_layers):
                pack, sched, idx_sb_d, m_d, src_tab, z_d, out_tab = layers[li]
                idx_sb = idxU_sb if idx_sb_d is idxU_d else idxV_sb
                T = pack.T
                is_last = (li == 2)
                # call index base per (window, stream)
                ci = 0
                tile_base = 0
                nchunk_w = nw // SUBS if nw % SUBS == 0 else None
                # windows per AG sub-chunk: distribute nw into SUBS groups
                wgroups = np.array_split(np.arange(nw), SUBS)

                for w in range(nw):
                    wt_lo, wt_hi = pack.wtiles[w]
                    wt = wt_lo + wt_hi
                    # gather calls for this window (lo then hi)
                    gtiles = []  # list of (gbuf, local offset) per tile
                    for s in range(2):
                        nt_s = pack.wtiles[w][s]
                        t0 = 0
                        while t0 < nt_s:
                            k, tb = sched[ci]
                            assert tb == tile_base + t0
                            gbuf = g_pool.tile([128, 8 * 128], bf, tag="g",
                                               name=f"g{li}")
                            nidx = k * 128
                            if nidx not in gather_regs:
                                gather_regs[nidx] = nc.gpsimd.to_reg(nidx)
                            src_ap = (src_tab[0:half, :] if s == 0
                                      else src_tab[half:RG, :])
                            nc.gpsimd.dma_gather(
                                gbuf[:, 0:k * 128].rearrange(
                                    "p (t e) -> p t e", e=128),
                                src_ap,
                                idx_sb[:, ci * 64:ci * 64 + k * 8],
                                num_idxs=nidx,
                                num_idxs_reg=gather_regs[nidx],
                                elem_size=128,
                                queue_num=call_no[0] % 4,
                            )
                            call_no[0] += 1
                            for j in range(k):
                                gtiles.append((gbuf, j))
                            ci += 1
                            t0 += k
                        tile_base += nt_s

                    # M slab for this window
                    mslab = m_pool.tile([128, 48 * PAGE], bf, tag="m")
                    mbase = tile_base - wt
                    nc.sync.dma_start(
                        out=mslab[:, 0:wt * PAGE],
                        in_=m_d[:, mbase * PAGE:(mbase + wt) * PAGE])

                    # PSUM bank for this window
                    pw = pagg_pool.tile([128, 512], f32, tag="pagg")
                    if is_last:
                        for pg in range(PAGES):
                            nc.tensor.matmul(
                                pw[0:PAGE, pg * 128:(pg + 1) * 128],
                                lhsT=V3_sb[:, w * WINDOW + pg * PAGE:
                                           w * WINDOW + (pg + 1) * PAGE],
                                rhs=C_sb[:],
                                start=True, stop=False, skip_group_check=True,
                            )
                    # MMs: tile order within window = (s, pg, t);
                    # PSUM region = page
                    started = [is_last] * PAGES
                    ti_w = 0
                    # per-page remaining MM counts to set stop flags
                    mm_left = [int(T[w, pg, 0] + T[w, pg, 1])
                               for pg in range(PAGES)]
                    for s in range(2):
                        for pg in range(PAGES):
                            for t in range(int(T[w, pg, s])):
                                gbuf, j = gtiles[ti_w]
                                mm_left[pg] -= 1
                                nc.tensor.matmul(
                                    pw[0:PAGE, pg * 128:(pg + 1) * 128],
                                    lhsT=mslab[:, ti_w * PAGE:(ti_w + 1) * PAGE],
                                    rhs=gbuf[:, j * 128:(j + 1) * 128],
                                    start=not started[pg],
                                    stop=(mm_left[pg] == 0),
                                    skip_group_check=True,
                                )
                                started[pg] = True
                                ti_w += 1
                    assert ti_w == wt

                    # copy bank -> staging
                    if is_last:
                        nc.vector.tensor_copy(
                            stag32[0:PAGE, w * 512:(w + 1) * 512],
                            pw[0:PAGE, :])
                    else:
                        nc.vector.tensor_copy(
                            stag[0:PAGE, w * 512:(w + 1) * 512],
                            pw[0:PAGE, :])

                    # z-write + sub-AG at group boundaries
                    if not is_last:
                        for sc in range(SUBS):
                            if w == wgroups[sc][-1]:
                                wlo = wgroups[sc][0]
                                k0, k1 = wlo * PAGES, (w + 1) * PAGES
                                dst = z_d.rearrange(
                                    "(k p) f -> p k f", p=PAGE)[:, k0:k1, :]
                                src = stag[0:PAGE, k0 * 128:k1 * 128].rearrange(
                                    "p (k f) -> p k f", f=128)
                                nc.sync.dma_start(out=dst, in_=src)
                                nc.gpsimd.collective_compute(
                                    "AllGather", mybir.AluOpType.bypass,
                                    replica_groups=[core_ids],
                                    ins=[z_d[wlo * WINDOW:(w + 1) * WINDOW, :]],
                                    outs=[out_tab[wlo * WINDOW * NC:
                                                  (w + 1) * WINDOW * NC, :]],
                                )

            # final output write
            if max_layers == 3:
                dst = out_d.rearrange("(k p) f -> p k f", p=PAGE)
                src = stag32[0:PAGE, :].rearrange("p (k f) -> p k f", f=128)
                nc.sync.dma_start(out=dst, in_=src)

    nc.compile()
    return nc


# ----------------------------------------------------------------------------
# public entry point
# ----------------------------------------------------------------------------


def kernel(**inputs) -> np.ndarray:
    from concourse.bass_utils import run_bass_kernel_spmd

    meta, per_core = prepare_host_data(inputs)
    nc = build_program(meta)
    res = run_bass_kernel_spmd(nc, per_core, list(range(NC)))

    n, loc = meta["n"], meta["loc"]
    posU = meta["posU"]
    out = np.zeros((n, 128), np.float32)
    for c in range(NC):
        g = np.arange(c * loc, (c + 1) * loc)
        out[g] = res.results[c]["outp"][posU[g]]
    return out


if __name__ == "__main__":
    data = dict(np.load("/root/problem/inputs_cache.npz"))
    got = kernel(**data)
    np.save("/root/problem/kernel_out.npy", got)
    print("kernel done", got.shape)
